# revision 27
# baseline (speedup 1.0000x reference)
"""Trainium2 Bass kernel for nn_BetaVAEMark7Decoder (v3).

All six layers are banded matmuls on the TensorEngine in bf16, data-parallel
over batch (4096 rows -> 512 per NeuronCore).  Biases ride as extra rows of
the stationary operands (activation tiles carry a constant ones-row; the f1
stage uses K=1 ones-stationary bias matmuls), so every PSUM evacuation is a
single bias-free leaky-relu instruction spread across the Scalar and Vector
engines.  fused2 evacuates full [128,512] tiles into x4 staging tiles; the
fused3 input windows are then built with SBUF-SBUF DMAs (idle DMA engines)
instead of fragmented partition-piece copies.  The final layer is blocked on
odd output-row boundaries (slot k = rows {2k+1,2k+2}) so each input slice
feeds exactly two PSUM slots.  Output is staged batch-major in bf16 as
(h, c4, w) and written with 8 large contiguous DMAs; the host transposes to
NCHW and upcasts to float32.
"""
import numpy as np
from contextlib import ExitStack

import concourse.bass as bass
import concourse.tile as tile
from concourse import bacc, mybir
from concourse.bass_utils import run_bass_kernel_spmd

F32 = mybir.dt.float32
BF16 = mybir.dt.bfloat16
AF = mybir.ActivationFunctionType
OP = mybir.AluOpType

NCORES = 8
BCORE = 512
CH = 128

# x1 windows: W_a holds wi in [w0, w0+4)
X1W = [(0, 4), (1, 4), (3, 4), (4, 4)]
# x3 (= post-fused1, j in 0..15, c2 in 0..15, split c2h halves) j-windows
X3_WIN = [(0, 9), (4, 12)]      # (j0, nj): A = j 0..8, B = j 4..15
# a3 (= x4, W in 0..31, c3 in 0..7) windows for fused3: (W0, nW)
A3_WIN = [(0, 15), (12, 15), (20, 12)]
# fused3 weight-col ownership (jc ranges) per window
A3_JC = [(0, 13), (13, 26), (26, 32)]


# ---------------- host-side weight factorization ----------------
def _precompute(w):
    P = {}
    w_lin, b_lin = w["w_lin"], w["b_lin"]
    lhs_lin = np.zeros((7, 256), np.float32)
    c_lin = np.zeros(256, np.float32)
    for wi in range(8):
        for ci in range(32):
            lhs_lin[:, wi * 32 + ci] = w_lin[:, ci * 8 + wi]
            c_lin[wi * 32 + ci] = b_lin[ci * 8 + wi]
    P["lhs_lin"], P["c_lin"] = lhs_lin, c_lin

    w_up1, b_up1, w_tc1, b_tc1 = w["w_up1"], w["b_up1"], w["w_tc1"], w["b_tc1"]
    K1 = np.zeros((5, 2, 3, 32, 16), np.float32)
    for hh in range(5):
        for s in range(2):
            for dh in range(3):
                hp = hh + 1 - dh
                if not (0 <= hp < 5):
                    continue
                for dw in range(3):
                    t = s + 1 - dw
                    dj = int(np.floor(t / 2))
                    kw = t - 2 * dj
                    K1[hh, s, dj + 1] += np.einsum("ic,cd->id", w_up1[hp, kw], w_tc1[dh, dw])
    c1 = np.zeros((5, 16, 16), np.float32)
    for hh in range(5):
        for ww in range(16):
            acc = b_tc1.copy()
            for dh in range(3):
                if not (0 <= hh + 1 - dh < 5):
                    continue
                for dw in range(3):
                    if not (0 <= ww + 1 - dw < 16):
                        continue
                    acc = acc + b_up1 @ w_tc1[dh, dw]
            c1[hh, ww] = acc
    P["K1"], P["c1"] = K1, c1

    w_up2, b_up2, w_tc2, b_tc2 = w["w_up2"], w["b_up2"], w["w_tc2"], w["b_tc2"]
    K2 = np.zeros((5, 2, 3, 3, 16, 8), np.float32)
    for r in range(5):
        for s in range(2):
            for dh in range(3):
                u = r + 1 - dh
                di = int(np.floor(u / 5))
                kh = u - 5 * di
                for dw in range(3):
                    t = s + 1 - dw
                    dj = int(np.floor(t / 2))
                    kw = t - 2 * dj
                    K2[r, s, di + 1, dj + 1] += np.einsum("ic,cd->id", w_up2[kh, kw], w_tc2[dh, dw])
    P["K2"] = K2
    P["BB2"] = np.einsum("c,hwcd->hwd", b_up2, w_tc2)
    P["b_tc2"] = b_tc2

    w_up3, b_up3, w_tc3, b_tc3 = w["w_up3"], w["b_up3"], w["w_tc3"], w["b_tc3"]
    K3 = np.zeros((2, 2, 3, 3, 8, 6), np.float32)
    for r in range(2):
        for s in range(2):
            for dh in range(3):
                u = r + 1 - dh
                di = int(np.floor(u / 2))
                kh = u - 2 * di
                for dw in range(3):
                    t = s + 1 - dw
                    dj = int(np.floor(t / 2))
                    kw = t - 2 * dj
                    K3[r, s, di + 1, dj + 1] += np.einsum("ic,cd->id", w_up3[kh, kw], w_tc3[dh, dw])
    P["K3"] = K3
    P["BB3"] = np.einsum("c,hwcd->hwd", b_up3, w_tc3)
    P["b_tc3"] = b_tc3
    return P


def _fused1_blocks(P):
    """Per (a = x3-j quad, hg = H group {0,1},{2,3},{4}): weight block
    [128, M] (rows = X1 window wi*32+ci) and bias block [1, M].
    Cols = (hi, c2h, wl, c2l)."""
    K1, c1 = P["K1"], P["c1"]
    blocks = {}
    for a in range(4):
        w0 = X1W[a][0]
        for hg in range(3):
            nh = 2 if hg < 2 else 1
            M = nh * 64
            B = np.zeros((128, M), np.float32)
            bias = np.zeros((1, M), np.float32)
            for hi in range(nh):
                hh = hg * 2 + hi
                for c2h in range(2):
                    for wl in range(4):
                        j = 4 * a + wl
                        ju, s = j // 2, j % 2
                        for c2l in range(8):
                            c2 = c2h * 8 + c2l
                            col = hi * 64 + c2h * 32 + wl * 8 + c2l
                            bias[0, col] = c1[hh, j, c2]
                            for wi_l in range(4):
                                wi = w0 + wi_l
                                dj = wi - ju
                                if -1 <= dj <= 1:
                                    B[wi_l * 32:(wi_l + 1) * 32, col] = K1[hh, s, dj + 1, :, c2]
            blocks[("f1w", a, hg)] = B
            blocks[("f1b", a, hg)] = bias
    return blocks


def _fused2_blocks(P):
    """Blocks per (half, r, c2h [, variants]): [K+1, 128] with rows =
    x3-window (j, c2l) and ones/bias row at K.  Cols = (wl 16, c3 8).
    c2h=0 main blocks carry the bias row; c2h=1 and halo blocks are zero."""
    K2, BB2, b_tc2 = P["K2"], P["BB2"], P["b_tc2"]

    def col_bias(Hh, Ww, c3):
        acc = b_tc2[c3]
        for dh in range(3):
            if not (0 <= Hh + 1 - dh < 25):
                continue
            for dw in range(3):
                if not (0 <= Ww + 1 - dw < 32):
                    continue
                acc += BB2[dh, dw, c3]
        return acc

    blocks = {}
    for half in range(2):
        win = 0 if half == 0 else 1
        j0, nj = X3_WIN[win]
        Kr = nj * 8

        def base(r, di, c2h):
            B = np.zeros((Kr + 1, 128), np.float32)
            for wl in range(16):
                Ww = 16 * half + wl
                j, s = Ww // 2, Ww % 2
                for c3 in range(8):
                    col = wl * 8 + c3
                    for jl in range(nj):
                        dj = (j0 + jl) - j
                        if -1 <= dj <= 1:
                            B[jl * 8:(jl + 1) * 8, col] = \
                                K2[r, s, di + 1, dj + 1, c2h * 8:(c2h + 1) * 8, c3]
            return B

        def add_bias(B, Hh):
            for wl in range(16):
                for c3 in range(8):
                    B[Kr, wl * 8 + c3] = col_bias(Hh, 16 * half + wl, c3)
            return B

        for r in range(5):
            for c2h in range(2):
                B = base(r, 0, c2h)
                if c2h == 1:
                    blocks[("f2", half, r, 1, "m")] = B
                    continue
                if r in (1, 2, 3):
                    blocks[("f2", half, r, 0, "m")] = add_bias(B.copy(), 5 + r)
                else:
                    edge_i = 0 if r == 0 else 4
                    blocks[("f2", half, r, 0, "mid")] = add_bias(B.copy(), 10 + r)
                    blocks[("f2", half, r, 0, "edge")] = add_bias(B.copy(), 5 * edge_i + r)
        for c2h in range(2):
            blocks[("f2", half, 0, c2h, "h")] = base(0, -1, c2h)
            blocks[("f2", half, 4, c2h, "h")] = base(4, 1, c2h)
    return blocks


def _fused3_blocks(P):
    """Slot-prime blocks.  Per window t: rows (W-W0)*8+c3, ones row at 8*nW.
    Cols ordered (q, c4, jc-own, s).  U: q=0 -> row 2i-1 (di=+1), q=1 -> row 2i
    (di=0, carries bias).  L: q=0 -> row 2i+1 (di=0, bias), q=1 -> row 2i+2
    (di=-1).  U0 = q=1 half with H'=0 edge bias; L24 = q=0 half, H'=49 edge."""
    K3, BB3, b_tc3 = P["K3"], P["BB3"], P["b_tc3"]

    def col_bias(jc, s, c4, drop_dh):
        acc = b_tc3[c4]
        for dh in range(3):
            if dh in drop_dh:
                continue
            for dw in range(3):
                tt = s + 1 - dw
                dj = int(np.floor(tt / 2))
                if 0 <= jc + dj < 32:
                    acc += BB3[dh, dw, c4]
        return acc

    blocks = {}
    for t in range(3):
        W0, nW = A3_WIN[t]
        lo, hi = A3_JC[t]
        njc = hi - lo
        K = 8 * nW
        half = 6 * njc * 2

        def fill(B, q, r, di, bias_drop=None):
            for c4 in range(6):
                for jci in range(njc):
                    jc = lo + jci
                    for s in range(2):
                        col = q * half + (c4 * njc + jci) * 2 + s
                        if bias_drop is not None:
                            B[K, col] = col_bias(jc, s, c4, bias_drop)
                        for dj in (-1, 0, 1):
                            j = jc + dj
                            if not (W0 <= j < W0 + nW) or not (0 <= j < 32):
                                continue
                            jl = j - W0
                            B[jl * 8:(jl + 1) * 8, col] = K3[r, s, di + 1, dj + 1, :, c4]

        U = np.zeros((K + 1, 2 * half), np.float32)
        fill(U, 0, 1, +1)                      # row 2i-1
        fill(U, 1, 0, 0, bias_drop=set())      # row 2i, interior bias
        blocks[("U", t)] = U
        L = np.zeros((K + 1, 2 * half), np.float32)
        fill(L, 0, 1, 0, bias_drop=set())      # row 2i+1, interior bias
        fill(L, 1, 0, -1)                      # row 2i+2
        blocks[("L", t)] = L
        U0 = np.zeros((K + 1, 2 * half), np.float32)
        fill(U0, 1, 0, 0, bias_drop={2})       # row 0 edge bias
        blocks[("U0", t)] = U0[:, half:]
        L24 = np.zeros((K + 1, 2 * half), np.float32)
        fill(L24, 0, 1, 0, bias_drop={0})      # row 49 edge bias
        blocks[("L24", t)] = L24[:, :half]
    return blocks


class _Pack:
    """Packs [K, M] blocks into one [128, cols] array; remembers offsets."""

    def __init__(self):
        self.cols = 0
        self.reg = {}
        self.items = []

    def add(self, key, arr):
        K, M = arr.shape
        self.reg[key] = (self.cols, K, M)
        self.items.append(arr)
        self.cols += M

    def build(self):
        out = np.zeros((128, self.cols), np.float32)
        c = 0
        for arr in self.items:
            K, M = arr.shape
            out[:K, c:c + M] = arr
            c += M
        return out


def _make_packs(inputs):
    P = _precompute(inputs)
    f1b = _fused1_blocks(P)
    f2b = _fused2_blocks(P)
    f3b = _fused3_blocks(P)

    wp = _Pack()
    lin0 = np.zeros((8, 128), np.float32)
    lin0[:7] = P["lhs_lin"][:, 0:128]
    lin0[7] = P["c_lin"][0:128]
    lin1 = np.zeros((8, 128), np.float32)
    lin1[:7] = P["lhs_lin"][:, 128:256]
    lin1[7] = P["c_lin"][128:256]
    wp.add("lin0", lin0)
    wp.add("lin1", lin1)
    for a in range(4):
        for hg in range(3):
            wp.add(("f1w", a, hg), f1b[("f1w", a, hg)])
            wp.add(("f1b", a, hg), f1b[("f1b", a, hg)])
    nearly = wp.cols
    for key, arr in f2b.items():
        wp.add(key, arr)
    for t in range(3):
        for kind in ("U", "L", "U0", "L24"):
            wp.add((kind, t), f3b[(kind, t)])
    return wp, nearly


# ---------------- device program ----------------
_PROG = {}


def _lim(s):
    s = s % 128
    if s == 0:
        return 128
    if s == 64:
        return 64
    return 32


def _pieces(p0, d0, n):
    assert p0 % 32 == 0 and d0 % 32 == 0, (p0, d0, n)
    out = []
    off = 0
    while off < n:
        s1, s2 = (p0 + off) % 128, (d0 + off) % 128
        c = min(n - off, _lim(s1), _lim(s2))
        out.append((off, c))
        off += c
    return out


def _build_program(wcols, nearly):
    key = (wcols, nearly)
    if key in _PROG:
        return _PROG[key]
    nc = bacc.Bacc("TRN2", target_bir_lowering=False, debug=False, num_devices=NCORES)
    lat_ap = nc.dram_tensor("latT", [8, BCORE], BF16, kind="ExternalInput").ap()
    wp_ap = nc.dram_tensor("wpack", [128, wcols], BF16, kind="ExternalInput").ap()
    ones_ap = nc.dram_tensor("ones", [1, 12800], BF16, kind="ExternalInput").ap()
    out_ap = nc.dram_tensor("out", [BCORE, 50, 6, 64], BF16, kind="ExternalOutput").ap()
    with tile.TileContext(nc) as tc:
        with ExitStack() as ctx:
            _emit(ctx, tc, nc, lat_ap, wp_ap, ones_ap, out_ap,
                  _build_program.wreg, nearly)
    nc.compile()
    _PROG[key] = nc
    return nc


def _emit(ctx, tc, nc, lat_ap, wp_ap, ones_ap, out_ap, wreg, nearly):
    wcols = wp_ap.shape[1]

    consts = ctx.enter_context(tc.tile_pool(name="consts", bufs=1))
    x1p = ctx.enter_context(tc.tile_pool(name="x1", bufs=1))
    x3p = ctx.enter_context(tc.tile_pool(name="x3", bufs=1))
    x4p0 = ctx.enter_context(tc.tile_pool(name="x4a", bufs=3))
    x4p1 = ctx.enter_context(tc.tile_pool(name="x4b", bufs=3))
    a3p = ctx.enter_context(tc.tile_pool(name="a3", bufs=1))
    stgp = ctx.enter_context(tc.tile_pool(name="stg", bufs=2))
    tmpp = ctx.enter_context(tc.tile_pool(name="tmp", bufs=4))
    pp128 = ctx.enter_context(tc.tile_pool(name="pp128", bufs=2, space="PSUM"))
    pA = ctx.enter_context(tc.tile_pool(name="pA", bufs=3, space="PSUM"))
    pB = ctx.enter_context(tc.tile_pool(name="pB", bufs=3, space="PSUM"))

    # ---- constants: split weight DMA so lin/f1 start early ----
    wbt = consts.tile([128, wcols], BF16)
    nc.sync.dma_start(wbt[:, 0:nearly], wp_ap[:, 0:nearly])
    nc.sync.dma_start(wbt[:, nearly:wcols], wp_ap[:, nearly:wcols])
    latT = consts.tile([8, BCORE], BF16)
    nc.sync.dma_start(latT[:], lat_ap[:])
    onesr = consts.tile([1, BCORE], BF16)
    nc.sync.dma_start(onesr[:], ones_ap[:, 0:BCORE])

    def W(key):
        o, K, M = wreg[key]
        return wbt[:K, o:o + M]

    # activation tiles
    x1t = [x1p.tile([128, BCORE], BF16, name=f"x1_{a}") for a in range(4)]
    # x3 tiles keyed (win, c2h, part); part 0 = H rows 0..1, part 1 = 2..4,
    # so fused2's early rows can start before fused1 finishes.  Ones row at
    # nj*8 in each tile.
    XP = [(0, 2), (2, 3)]  # (H base, nH) per part
    x3t = {}
    for win in range(2):
        for c2h in range(2):
            Kr = X3_WIN[win][1] * 8
            for part, (hb, nH) in enumerate(XP):
                t_ = x3p.tile([Kr + 1, nH * BCORE], BF16,
                              name=f"x3_{win}_{c2h}_{part}")
                x3t[(win, c2h, part)] = t_
    a3K = [A3_WIN[t][1] * 8 for t in range(3)]
    a3t = [a3p.tile([a3K[t] + 1, 25 * BCORE], BF16, name=f"a3_{t}") for t in range(3)]

    # ---- evac dispatcher: lrelu(psum) -> dst ----
    ev_n = [0]

    def lrelu_evac(dst, ps, np_, nf):
        k = ev_n[0]
        ev_n[0] += 1
        m = k % 8
        if m < 5:
            nc.scalar.activation(dst, ps, AF.Lrelu, bias=0.0, scale=1.0, alpha=0.01)
        else:
            tmp = tmpp.tile([128, 512], BF16, tag="evt", name=f"evt{k}")
            tv = tmp[0:np_, 0:nf]
            nc.vector.tensor_copy(tv, ps)
            nc.vector.scalar_tensor_tensor(dst, tv, 0.01, tv, op0=OP.mult, op1=OP.max)

    # ---- lin -> x1 windows ----
    psA = pp128.tile([128, BCORE], F32, tag="p128", name="lin0")
    nc.tensor.matmul(psA[:], W("lin0"), latT[:], start=True, stop=True)
    psB = pp128.tile([128, BCORE], F32, tag="p128", name="lin1")
    nc.tensor.matmul(psB[:], W("lin1"), latT[:], start=True, stop=True)
    for a in range(4):
        w0 = X1W[a][0]
        for ps, base in ((psA, 0), (psB, 4)):
            lo = max(w0, base)
            hi = min(w0 + 4, base + 4)
            if lo >= hi:
                continue
            p0, d0, n = (lo - base) * 32, (lo - w0) * 32, (hi - lo) * 32
            for off, cnt in _pieces(p0, d0, n):
                lrelu_evac(x1t[a][d0 + off:d0 + off + cnt, :],
                           ps[p0 + off:p0 + off + cnt, :], cnt, BCORE)

    # ---- fused1 -> x3 (c2h-halved windows) ----
    # fill ownership: window A owns j 0..3, B owns j 4..15
    for a in range(4):
        for hg in range(3):
            nh = 2 if hg < 2 else 1
            M = nh * 64
            ps = pp128.tile([128, BCORE], F32, tag="p128", name=f"f1_{a}_{hg}")
            nc.tensor.matmul(ps[0:M, :], W(("f1w", a, hg)), x1t[a][:],
                             start=True, stop=False)
            nc.tensor.matmul(ps[0:M, :], W(("f1b", a, hg)), onesr[:],
                             start=False, stop=True)
            win = 0 if a == 0 else 1
            j0 = X3_WIN[win][0]
            d0 = (4 * a - j0) * 8
            for hi in range(nh):
                hh = hg * 2 + hi
                part = 0 if hh < 2 else 1
                fsl = slice((hh - XP[part][0]) * BCORE,
                            (hh - XP[part][0] + 1) * BCORE)
                for c2h in range(2):
                    p0 = hi * 64 + c2h * 32
                    lrelu_evac(x3t[(win, c2h, part)][d0:d0 + 32, fsl],
                               ps[p0:p0 + 32, :], 32, BCORE)

    # x3 ones rows (deferred emission: keep early DMA queue for inputs)
    for win in range(2):
        Kr = X3_WIN[win][1] * 8
        for c2h in range(2):
            for part, (hb, nH) in enumerate(XP):
                nc.sync.dma_start(x3t[(win, c2h, part)][Kr:Kr + 1, :],
                                  ones_ap[:, 0:nH * BCORE])
    # x3 mirror: window A rows j 4..8 <- window B rows 0..40
    for c2h in range(2):
        for part in range(2):
            nc.sync.dma_start(x3t[(0, c2h, part)][32:72, :],
                              x3t[(1, c2h, part)][0:40, :])

    # ---- fused2 -> x4 staging -> a3 windows via DMA ----
    def xmv(half, c2h, i):
        win = 0 if half == 0 else 1
        Kr = X3_WIN[win][1] * 8
        part = 0 if i < 2 else 1
        b0 = (i - XP[part][0]) * BCORE
        return x3t[(win, c2h, part)][0:Kr + 1, b0:b0 + BCORE]

    # a3 ones rows (deferred so the lin/f1 input DMAs go first on the queue)
    for t in range(3):
        nc.sync.dma_start(a3t[t][a3K[t]:a3K[t] + 1, :], ones_ap[:, 0:25 * BCORE])

    for i in range(5):
        x4 = [x4p0.tile([128, 5 * BCORE], BF16, tag="x4a", name=f"x4_{i}_0"),
              x4p1.tile([128, 5 * BCORE], BF16, tag="x4b", name=f"x4_{i}_1")]
        for half in range(2):
            for r in range(5):
                ps = pp128.tile([128, BCORE], F32, tag="p128", name=f"f2_{i}_{half}_{r}")
                if r in (1, 2, 3):
                    tag0 = "m"
                else:
                    edge_i = 0 if r == 0 else 4
                    tag0 = "edge" if i == edge_i else "mid"
                nc.tensor.matmul(ps[:], W(("f2", half, r, 0, tag0)),
                                 xmv(half, 0, i), start=True, stop=False)
                last = r in (1, 2, 3) or i == (0 if r == 0 else 4)
                nc.tensor.matmul(ps[:], W(("f2", half, r, 1, "m")),
                                 xmv(half, 1, i), start=False, stop=last)
                if r == 0 and i > 0:
                    nc.tensor.matmul(ps[:], W(("f2", half, 0, 0, "h")),
                                     xmv(half, 0, i - 1), start=False, stop=False)
                    nc.tensor.matmul(ps[:], W(("f2", half, 0, 1, "h")),
                                     xmv(half, 1, i - 1), start=False, stop=True)
                if r == 4 and i < 4:
                    nc.tensor.matmul(ps[:], W(("f2", half, 4, 0, "h")),
                                     xmv(half, 0, i + 1), start=False, stop=False)
                    nc.tensor.matmul(ps[:], W(("f2", half, 4, 1, "h")),
                                     xmv(half, 1, i + 1), start=False, stop=True)
                lrelu_evac(x4[half][:, r * BCORE:(r + 1) * BCORE], ps[:], 128, BCORE)
        # build a3 window slices for H rows 5i..5i+4 (cols i*2560..);
        # issued from the idle GpSimd SWDGE queue to keep Sync free
        csl = slice(i * 5 * BCORE, (i + 1) * 5 * BCORE)
        nc.gpsimd.dma_start(a3t[0][0:120, csl], x4[0][0:120, :])
        nc.gpsimd.dma_start(a3t[1][0:32, csl], x4[0][96:128, :])
        nc.gpsimd.dma_start(a3t[1][32:120, csl], x4[1][0:88, :])
        nc.gpsimd.dma_start(a3t[2][0:96, csl], x4[1][32:128, :])

    # ---- fused3 ----
    NT = [2 * 6 * (A3_JC[t][1] - A3_JC[t][0]) * 2 for t in range(3)]  # 312,312,144

    def st_ap(t, i, c):
        return a3t[t][0:a3K[t] + 1, i * BCORE + c * CH:i * BCORE + (c + 1) * CH]

    for c in range(4):
        slot = {}

        def slot_tiles(k):
            if k not in slot:
                ta = pA.tile([128, NT[0]], F32, tag="pA", name=f"sA_{c}_{k}")
                tb = pB.tile([128, NT[1] + NT[2]], F32, tag="pB", name=f"sB_{c}_{k}")
                slot[k] = (ta, tb)
            return slot[k]

        def regions(k):
            ta, tb = slot_tiles(k)
            return [ta[:, 0:NT[0]], tb[:, 0:NT[1]], tb[:, NT[1]:NT[1] + NT[2]]]

        stg = None

        def evac_slot(k):
            rlo = max(2 * k + 1, 0)
            rhi = min(2 * k + 2, 49)
            half = 0 if rhi < 25 else 1
            hb = 25 * half
            stgw = stg[:].rearrange("p (h c4 jc s) -> p h c4 jc s",
                                    h=25, c4=6, jc=32, s=2)
            for t in range(3):
                lo, hi = A3_JC[t]
                reg = regions(k)[t]
                ht = NT[t] // 2
                if 2 * k + 1 < 0:       # slot -1: row 0 only (q=1 half)
                    src = reg[:, ht:NT[t]]
                    dst = stgw[:, 0:1, :, lo:hi, :]
                elif 2 * k + 2 > 49:    # slot 24: row 49 only (q=0 half)
                    src = reg[:, 0:ht]
                    dst = stgw[:, 49 - hb:50 - hb, :, lo:hi, :]
                else:
                    src = reg[:]
                    dst = stgw[:, rlo - hb:rhi + 1 - hb, :, lo:hi, :]
                lrelu_evac(dst, src, 128, src.shape[1])

        for i in range(25):
            if i == 0:
                stg = stgp.tile([128, 9600], BF16, tag="stg", name=f"stg_{c}_0")
            for t in range(3):
                st = st_ap(t, i, c)
                # PSUM start=True marks the whole 2KB bank pending-zero, so
                # only the first write into each bank per slot-generation may
                # carry it; t=2 shares the pB bank with t=1.
                first_in_bank = t != 2
                if i == 0:
                    reg = regions(-1)[t]
                    ht = NT[t] // 2
                    nc.tensor.matmul(reg[:, ht:NT[t]], st, W(("U0", t)),
                                     start=first_in_bank, stop=True,
                                     skip_group_check=True)
                else:
                    reg = regions(i - 1)[t]
                    nc.tensor.matmul(reg[:], st, W(("U", t)),
                                     start=False, stop=True, skip_group_check=True)
                if i == 24:
                    reg = regions(24)[t]
                    ht = NT[t] // 2
                    nc.tensor.matmul(reg[:, 0:ht], st, W(("L24", t)),
                                     start=first_in_bank, stop=True,
                                     skip_group_check=True)
                else:
                    reg = regions(i)[t]
                    nc.tensor.matmul(reg[:], st, W(("L", t)),
                                     start=first_in_bank, stop=False,
                                     skip_group_check=True)
            evac_slot(i - 1)
            del slot[i - 1]
            if i - 1 == 11:  # rows 0..24 complete -> flush half 0
                sv = stg[:].rearrange("p (h c4 w) -> p h c4 w", h=25, c4=6, w=64)
                nc.sync.dma_start(out_ap[c * CH:(c + 1) * CH, 0:25, :, :], sv)
                stg = stgp.tile([128, 9600], BF16, tag="stg", name=f"stg_{c}_1")
        evac_slot(24)
        sv = stg[:].rearrange("p (h c4 w) -> p h c4 w", h=25, c4=6, w=64)
        nc.sync.dma_start(out_ap[c * CH:(c + 1) * CH, 25:50, :, :], sv)


def build_inmaps(inputs):
    import ml_dtypes
    wp, nearly = _make_packs(inputs)
    wpack = wp.build().astype(ml_dtypes.bfloat16)
    _build_program.wreg = wp.reg
    lat = np.asarray(inputs["latent"], np.float32)
    ones = np.ones((1, 12800), ml_dtypes.bfloat16)
    in_maps = []
    for i in range(NCORES):
        latT = np.ones((8, BCORE), np.float32)
        latT[:7] = lat[i * BCORE:(i + 1) * BCORE].T
        in_maps.append({"latT": latT.astype(ml_dtypes.bfloat16), "wpack": wpack,
                        "ones": ones})
    return in_maps, wpack.shape[1], nearly


def kernel(**inputs):
    inputs = {k: np.asarray(v) for k, v in inputs.items()}
    in_maps, wcols, nearly = build_inmaps(inputs)
    nc = _build_program(wcols, nearly)
    res = run_bass_kernel_spmd(nc, in_maps, core_ids=list(range(NCORES)))
    out = np.concatenate([np.asarray(res.results[i]["out"]) for i in range(NCORES)],
                         axis=0)
    return out.transpose(0, 2, 1, 3).astype(np.float32)


# revision 28
# speedup vs baseline: 1.0196x; 1.0196x over previous
"""Trainium2 Bass kernel for nn_BetaVAEMark7Decoder (v3).

All six layers are banded matmuls on the TensorEngine in bf16, data-parallel
over batch (4096 rows -> 512 per NeuronCore).  Biases ride as extra rows of
the stationary operands (activation tiles carry a constant ones-row; the f1
stage uses K=1 ones-stationary bias matmuls), so every PSUM evacuation is a
single bias-free leaky-relu instruction spread across the Scalar and Vector
engines.  fused2 evacuates full [128,512] tiles into x4 staging tiles; the
fused3 input windows are then built with SBUF-SBUF DMAs (idle DMA engines)
instead of fragmented partition-piece copies.  The final layer is blocked on
odd output-row boundaries (slot k = rows {2k+1,2k+2}) so each input slice
feeds exactly two PSUM slots.  Output is staged batch-major in bf16 as
(h, c4, w) and written with 8 large contiguous DMAs; the host transposes to
NCHW and upcasts to float32.
"""
import numpy as np
from contextlib import ExitStack

import concourse.bass as bass
import concourse.tile as tile
from concourse import bacc, mybir
from concourse.bass_utils import run_bass_kernel_spmd

F32 = mybir.dt.float32
BF16 = mybir.dt.bfloat16
AF = mybir.ActivationFunctionType
OP = mybir.AluOpType

NCORES = 8
BCORE = 512
CH = 128

# x1 windows: W_a holds wi in [w0, w0+4)
X1W = [(0, 4), (1, 4), (3, 4), (4, 4)]
# x3 (= post-fused1, j in 0..15, c2 in 0..15, split c2h halves) j-windows
X3_WIN = [(0, 9), (4, 12)]      # (j0, nj): A = j 0..8, B = j 4..15
# a3 (= x4, W in 0..31, c3 in 0..7) windows for fused3: (W0, nW)
A3_WIN = [(0, 15), (12, 15), (20, 12)]
# fused3 weight-col ownership (jc ranges) per window
A3_JC = [(0, 13), (13, 26), (26, 32)]


# ---------------- host-side weight factorization ----------------
def _precompute(w):
    P = {}
    w_lin, b_lin = w["w_lin"], w["b_lin"]
    lhs_lin = np.zeros((7, 256), np.float32)
    c_lin = np.zeros(256, np.float32)
    for wi in range(8):
        for ci in range(32):
            lhs_lin[:, wi * 32 + ci] = w_lin[:, ci * 8 + wi]
            c_lin[wi * 32 + ci] = b_lin[ci * 8 + wi]
    P["lhs_lin"], P["c_lin"] = lhs_lin, c_lin

    w_up1, b_up1, w_tc1, b_tc1 = w["w_up1"], w["b_up1"], w["w_tc1"], w["b_tc1"]
    K1 = np.zeros((5, 2, 3, 32, 16), np.float32)
    for hh in range(5):
        for s in range(2):
            for dh in range(3):
                hp = hh + 1 - dh
                if not (0 <= hp < 5):
                    continue
                for dw in range(3):
                    t = s + 1 - dw
                    dj = int(np.floor(t / 2))
                    kw = t - 2 * dj
                    K1[hh, s, dj + 1] += np.einsum("ic,cd->id", w_up1[hp, kw], w_tc1[dh, dw])
    c1 = np.zeros((5, 16, 16), np.float32)
    for hh in range(5):
        for ww in range(16):
            acc = b_tc1.copy()
            for dh in range(3):
                if not (0 <= hh + 1 - dh < 5):
                    continue
                for dw in range(3):
                    if not (0 <= ww + 1 - dw < 16):
                        continue
                    acc = acc + b_up1 @ w_tc1[dh, dw]
            c1[hh, ww] = acc
    P["K1"], P["c1"] = K1, c1

    w_up2, b_up2, w_tc2, b_tc2 = w["w_up2"], w["b_up2"], w["w_tc2"], w["b_tc2"]
    K2 = np.zeros((5, 2, 3, 3, 16, 8), np.float32)
    for r in range(5):
        for s in range(2):
            for dh in range(3):
                u = r + 1 - dh
                di = int(np.floor(u / 5))
                kh = u - 5 * di
                for dw in range(3):
                    t = s + 1 - dw
                    dj = int(np.floor(t / 2))
                    kw = t - 2 * dj
                    K2[r, s, di + 1, dj + 1] += np.einsum("ic,cd->id", w_up2[kh, kw], w_tc2[dh, dw])
    P["K2"] = K2
    P["BB2"] = np.einsum("c,hwcd->hwd", b_up2, w_tc2)
    P["b_tc2"] = b_tc2

    w_up3, b_up3, w_tc3, b_tc3 = w["w_up3"], w["b_up3"], w["w_tc3"], w["b_tc3"]
    K3 = np.zeros((2, 2, 3, 3, 8, 6), np.float32)
    for r in range(2):
        for s in range(2):
            for dh in range(3):
                u = r + 1 - dh
                di = int(np.floor(u / 2))
                kh = u - 2 * di
                for dw in range(3):
                    t = s + 1 - dw
                    dj = int(np.floor(t / 2))
                    kw = t - 2 * dj
                    K3[r, s, di + 1, dj + 1] += np.einsum("ic,cd->id", w_up3[kh, kw], w_tc3[dh, dw])
    P["K3"] = K3
    P["BB3"] = np.einsum("c,hwcd->hwd", b_up3, w_tc3)
    P["b_tc3"] = b_tc3
    return P


def _fused1_blocks(P):
    """Per (a = x3-j quad, hg = H group {0,1},{2,3},{4}): weight block
    [128, M] (rows = X1 window wi*32+ci) and bias block [1, M].
    Cols = (hi, c2h, wl, c2l)."""
    K1, c1 = P["K1"], P["c1"]
    blocks = {}
    for a in range(4):
        w0 = X1W[a][0]
        for hg in range(3):
            nh = 2 if hg < 2 else 1
            M = nh * 64
            B = np.zeros((128, M), np.float32)
            bias = np.zeros((1, M), np.float32)
            for hi in range(nh):
                hh = hg * 2 + hi
                for c2h in range(2):
                    for wl in range(4):
                        j = 4 * a + wl
                        ju, s = j // 2, j % 2
                        for c2l in range(8):
                            c2 = c2h * 8 + c2l
                            col = hi * 64 + c2h * 32 + wl * 8 + c2l
                            bias[0, col] = c1[hh, j, c2]
                            for wi_l in range(4):
                                wi = w0 + wi_l
                                dj = wi - ju
                                if -1 <= dj <= 1:
                                    B[wi_l * 32:(wi_l + 1) * 32, col] = K1[hh, s, dj + 1, :, c2]
            blocks[("f1w", a, hg)] = B
            blocks[("f1b", a, hg)] = bias
    return blocks


def _fused2_blocks(P):
    """Blocks per (half, r, c2h [, variants]): [K+1, 128] with rows =
    x3-window (j, c2l) and ones/bias row at K.  Cols = (wl 16, c3 8).
    c2h=0 main blocks carry the bias row; c2h=1 and halo blocks are zero."""
    K2, BB2, b_tc2 = P["K2"], P["BB2"], P["b_tc2"]

    def col_bias(Hh, Ww, c3):
        acc = b_tc2[c3]
        for dh in range(3):
            if not (0 <= Hh + 1 - dh < 25):
                continue
            for dw in range(3):
                if not (0 <= Ww + 1 - dw < 32):
                    continue
                acc += BB2[dh, dw, c3]
        return acc

    blocks = {}
    for half in range(2):
        win = 0 if half == 0 else 1
        j0, nj = X3_WIN[win]
        Kr = nj * 8

        def base(r, di, c2h):
            B = np.zeros((Kr + 1, 128), np.float32)
            for wl in range(16):
                Ww = 16 * half + wl
                j, s = Ww // 2, Ww % 2
                for c3 in range(8):
                    col = wl * 8 + c3
                    for jl in range(nj):
                        dj = (j0 + jl) - j
                        if -1 <= dj <= 1:
                            B[jl * 8:(jl + 1) * 8, col] = \
                                K2[r, s, di + 1, dj + 1, c2h * 8:(c2h + 1) * 8, c3]
            return B

        def add_bias(B, Hh):
            for wl in range(16):
                for c3 in range(8):
                    B[Kr, wl * 8 + c3] = col_bias(Hh, 16 * half + wl, c3)
            return B

        for r in range(5):
            for c2h in range(2):
                B = base(r, 0, c2h)
                if c2h == 1:
                    blocks[("f2", half, r, 1, "m")] = B
                    continue
                if r in (1, 2, 3):
                    blocks[("f2", half, r, 0, "m")] = add_bias(B.copy(), 5 + r)
                else:
                    edge_i = 0 if r == 0 else 4
                    blocks[("f2", half, r, 0, "mid")] = add_bias(B.copy(), 10 + r)
                    blocks[("f2", half, r, 0, "edge")] = add_bias(B.copy(), 5 * edge_i + r)
        for c2h in range(2):
            blocks[("f2", half, 0, c2h, "h")] = base(0, -1, c2h)
            blocks[("f2", half, 4, c2h, "h")] = base(4, 1, c2h)
    return blocks


def _fused3_blocks(P):
    """Slot-prime blocks.  Per window t: rows (W-W0)*8+c3, ones row at 8*nW.
    Cols ordered (q, c4, jc-own, s).  U: q=0 -> row 2i-1 (di=+1), q=1 -> row 2i
    (di=0, carries bias).  L: q=0 -> row 2i+1 (di=0, bias), q=1 -> row 2i+2
    (di=-1).  U0 = q=1 half with H'=0 edge bias; L24 = q=0 half, H'=49 edge."""
    K3, BB3, b_tc3 = P["K3"], P["BB3"], P["b_tc3"]

    def col_bias(jc, s, c4, drop_dh):
        acc = b_tc3[c4]
        for dh in range(3):
            if dh in drop_dh:
                continue
            for dw in range(3):
                tt = s + 1 - dw
                dj = int(np.floor(tt / 2))
                if 0 <= jc + dj < 32:
                    acc += BB3[dh, dw, c4]
        return acc

    blocks = {}
    for t in range(3):
        W0, nW = A3_WIN[t]
        lo, hi = A3_JC[t]
        njc = hi - lo
        K = 8 * nW
        half = 6 * njc * 2

        def fill(B, q, r, di, bias_drop=None):
            for c4 in range(6):
                for jci in range(njc):
                    jc = lo + jci
                    for s in range(2):
                        col = q * half + (c4 * njc + jci) * 2 + s
                        if bias_drop is not None:
                            B[K, col] = col_bias(jc, s, c4, bias_drop)
                        for dj in (-1, 0, 1):
                            j = jc + dj
                            if not (W0 <= j < W0 + nW) or not (0 <= j < 32):
                                continue
                            jl = j - W0
                            B[jl * 8:(jl + 1) * 8, col] = K3[r, s, di + 1, dj + 1, :, c4]

        U = np.zeros((K + 1, 2 * half), np.float32)
        fill(U, 0, 1, +1)                      # row 2i-1
        fill(U, 1, 0, 0, bias_drop=set())      # row 2i, interior bias
        blocks[("U", t)] = U
        L = np.zeros((K + 1, 2 * half), np.float32)
        fill(L, 0, 1, 0, bias_drop=set())      # row 2i+1, interior bias
        fill(L, 1, 0, -1)                      # row 2i+2
        blocks[("L", t)] = L
        U0 = np.zeros((K + 1, 2 * half), np.float32)
        fill(U0, 1, 0, 0, bias_drop={2})       # row 0 edge bias
        blocks[("U0", t)] = U0[:, half:]
        L24 = np.zeros((K + 1, 2 * half), np.float32)
        fill(L24, 0, 1, 0, bias_drop={0})      # row 49 edge bias
        blocks[("L24", t)] = L24[:, :half]
    return blocks


class _Pack:
    """Packs [K, M] blocks into one [128, cols] array; remembers offsets."""

    def __init__(self):
        self.cols = 0
        self.reg = {}
        self.items = []

    def add(self, key, arr):
        K, M = arr.shape
        self.reg[key] = (self.cols, K, M)
        self.items.append(arr)
        self.cols += M

    def build(self):
        out = np.zeros((128, self.cols), np.float32)
        c = 0
        for arr in self.items:
            K, M = arr.shape
            out[:K, c:c + M] = arr
            c += M
        return out


def _make_packs(inputs):
    P = _precompute(inputs)
    f1b = _fused1_blocks(P)
    f2b = _fused2_blocks(P)
    f3b = _fused3_blocks(P)

    wp = _Pack()
    lin0 = np.zeros((8, 128), np.float32)
    lin0[:7] = P["lhs_lin"][:, 0:128]
    lin0[7] = P["c_lin"][0:128]
    lin1 = np.zeros((8, 128), np.float32)
    lin1[:7] = P["lhs_lin"][:, 128:256]
    lin1[7] = P["c_lin"][128:256]
    wp.add("lin0", lin0)
    wp.add("lin1", lin1)
    for a in range(4):
        for hg in range(3):
            wp.add(("f1w", a, hg), f1b[("f1w", a, hg)])
            wp.add(("f1b", a, hg), f1b[("f1b", a, hg)])
    nearly = wp.cols
    for key, arr in f2b.items():
        wp.add(key, arr)
    for t in range(3):
        for kind in ("U", "L", "U0", "L24"):
            wp.add((kind, t), f3b[(kind, t)])
    return wp, nearly


# ---------------- device program ----------------
_PROG = {}


def _lim(s):
    s = s % 128
    if s == 0:
        return 128
    if s == 64:
        return 64
    return 32


def _pieces(p0, d0, n):
    assert p0 % 32 == 0 and d0 % 32 == 0, (p0, d0, n)
    out = []
    off = 0
    while off < n:
        s1, s2 = (p0 + off) % 128, (d0 + off) % 128
        c = min(n - off, _lim(s1), _lim(s2))
        out.append((off, c))
        off += c
    return out


def _build_program(wcols, nearly):
    key = (wcols, nearly)
    if key in _PROG:
        return _PROG[key]
    nc = bacc.Bacc("TRN2", target_bir_lowering=False, debug=False, num_devices=NCORES)
    lat_ap = nc.dram_tensor("latT", [8, BCORE], BF16, kind="ExternalInput").ap()
    wp_ap = nc.dram_tensor("wpack", [128, wcols], BF16, kind="ExternalInput").ap()
    ones_ap = nc.dram_tensor("ones", [1, 12800], BF16, kind="ExternalInput").ap()
    out_ap = nc.dram_tensor("out", [BCORE, 50, 6, 64], BF16, kind="ExternalOutput").ap()
    with tile.TileContext(nc) as tc:
        with ExitStack() as ctx:
            _emit(ctx, tc, nc, lat_ap, wp_ap, ones_ap, out_ap,
                  _build_program.wreg, nearly)
    nc.compile()
    _PROG[key] = nc
    return nc


def _emit(ctx, tc, nc, lat_ap, wp_ap, ones_ap, out_ap, wreg, nearly):
    wcols = wp_ap.shape[1]

    consts = ctx.enter_context(tc.tile_pool(name="consts", bufs=1))
    x1p = ctx.enter_context(tc.tile_pool(name="x1", bufs=1))
    x3p = ctx.enter_context(tc.tile_pool(name="x3", bufs=1))
    x4p0 = ctx.enter_context(tc.tile_pool(name="x4a", bufs=3))
    x4p1 = ctx.enter_context(tc.tile_pool(name="x4b", bufs=3))
    a3p = ctx.enter_context(tc.tile_pool(name="a3", bufs=1))
    stgp = ctx.enter_context(tc.tile_pool(name="stg", bufs=2))
    tmpp = ctx.enter_context(tc.tile_pool(name="tmp", bufs=4))
    ps_ctx = ExitStack()
    pp128 = ps_ctx.enter_context(tc.tile_pool(name="pp128", bufs=6, space="PSUM"))

    # ---- constants: split weight DMA so lin/f1 start early ----
    wbt = consts.tile([128, wcols], BF16)
    nc.sync.dma_start(wbt[:, 0:nearly], wp_ap[:, 0:nearly])
    nc.sync.dma_start(wbt[:, nearly:wcols], wp_ap[:, nearly:wcols])
    latT = consts.tile([8, BCORE], BF16)
    nc.sync.dma_start(latT[:], lat_ap[:])
    onesr = consts.tile([1, BCORE], BF16)
    nc.sync.dma_start(onesr[:], ones_ap[:, 0:BCORE])

    def W(key):
        o, K, M = wreg[key]
        return wbt[:K, o:o + M]

    # activation tiles
    x1t = [x1p.tile([128, BCORE], BF16, name=f"x1_{a}") for a in range(4)]
    # x3 tiles keyed (win, c2h); ones row at nj*8
    x3t = {}
    for win in range(2):
        for c2h in range(2):
            Kr = X3_WIN[win][1] * 8
            t_ = x3p.tile([Kr + 1, 5 * BCORE], BF16, name=f"x3_{win}_{c2h}")
            x3t[(win, c2h)] = t_
            nc.sync.dma_start(t_[Kr:Kr + 1, :], ones_ap[:, 0:5 * BCORE])
    a3K = [A3_WIN[t][1] * 8 for t in range(3)]
    a3t = [a3p.tile([a3K[t] + 1, 25 * BCORE], BF16, name=f"a3_{t}") for t in range(3)]

    # ---- evac dispatcher: lrelu(psum) -> dst ----
    ev_n = [0]

    def lrelu_evac(dst, ps, np_, nf):
        k = ev_n[0]
        ev_n[0] += 1
        m = k % 8
        if m < 5:
            nc.scalar.activation(dst, ps, AF.Lrelu, bias=0.0, scale=1.0, alpha=0.01)
        else:
            tmp = tmpp.tile([128, 512], BF16, tag="evt", name=f"evt{k}")
            tv = tmp[0:np_, 0:nf]
            nc.vector.tensor_copy(tv, ps)
            nc.vector.scalar_tensor_tensor(dst, tv, 0.01, tv, op0=OP.mult, op1=OP.max)

    # ---- lin -> x1 windows ----
    psA = pp128.tile([128, BCORE], F32, tag="p128", name="lin0")
    nc.tensor.matmul(psA[:], W("lin0"), latT[:], start=True, stop=True)
    psB = pp128.tile([128, BCORE], F32, tag="p128", name="lin1")
    nc.tensor.matmul(psB[:], W("lin1"), latT[:], start=True, stop=True)
    for a in range(4):
        w0 = X1W[a][0]
        for ps, base in ((psA, 0), (psB, 4)):
            lo = max(w0, base)
            hi = min(w0 + 4, base + 4)
            if lo >= hi:
                continue
            p0, d0, n = (lo - base) * 32, (lo - w0) * 32, (hi - lo) * 32
            for off, cnt in _pieces(p0, d0, n):
                lrelu_evac(x1t[a][d0 + off:d0 + off + cnt, :],
                           ps[p0 + off:p0 + off + cnt, :], cnt, BCORE)

    # ---- fused1 -> x3 (c2h-halved windows) ----
    # fill ownership: window A owns j 0..3, B owns j 4..15
    for a in range(4):
        for hg in range(3):
            nh = 2 if hg < 2 else 1
            M = nh * 64
            ps = pp128.tile([128, BCORE], F32, tag="p128", name=f"f1_{a}_{hg}")
            nc.tensor.matmul(ps[0:M, :], W(("f1w", a, hg)), x1t[a][:],
                             start=True, stop=False)
            nc.tensor.matmul(ps[0:M, :], W(("f1b", a, hg)), onesr[:],
                             start=False, stop=True)
            win = 0 if a == 0 else 1
            j0 = X3_WIN[win][0]
            d0 = (4 * a - j0) * 8
            for hi in range(nh):
                hh = hg * 2 + hi
                fsl = slice(hh * BCORE, (hh + 1) * BCORE)
                for c2h in range(2):
                    p0 = hi * 64 + c2h * 32
                    lrelu_evac(x3t[(win, c2h)][d0:d0 + 32, fsl],
                               ps[p0:p0 + 32, :], 32, BCORE)

    # x3 mirror: window A rows j 4..8 <- window B rows 0..40
    for c2h in range(2):
        nc.sync.dma_start(x3t[(0, c2h)][32:72, :], x3t[(1, c2h)][0:40, :])

    # ---- fused2 -> x4 staging -> a3 windows via DMA ----
    def xmv(half, c2h, i):
        win = 0 if half == 0 else 1
        Kr = X3_WIN[win][1] * 8
        return x3t[(win, c2h)][0:Kr + 1, i * BCORE:(i + 1) * BCORE]

    # a3 ones rows (deferred so the lin/f1 input DMAs go first on the queue)
    for t in range(3):
        nc.sync.dma_start(a3t[t][a3K[t]:a3K[t] + 1, :], ones_ap[:, 0:25 * BCORE])

    for i in range(5):
        x4 = [x4p0.tile([128, 5 * BCORE], BF16, tag="x4a", name=f"x4_{i}_0"),
              x4p1.tile([128, 5 * BCORE], BF16, tag="x4b", name=f"x4_{i}_1")]
        for half in range(2):
            for r in range(5):
                ps = pp128.tile([128, BCORE], F32, tag="p128", name=f"f2_{i}_{half}_{r}")
                if r in (1, 2, 3):
                    tag0 = "m"
                else:
                    edge_i = 0 if r == 0 else 4
                    tag0 = "edge" if i == edge_i else "mid"
                nc.tensor.matmul(ps[:], W(("f2", half, r, 0, tag0)),
                                 xmv(half, 0, i), start=True, stop=False)
                last = r in (1, 2, 3) or i == (0 if r == 0 else 4)
                nc.tensor.matmul(ps[:], W(("f2", half, r, 1, "m")),
                                 xmv(half, 1, i), start=False, stop=last)
                if r == 0 and i > 0:
                    nc.tensor.matmul(ps[:], W(("f2", half, 0, 0, "h")),
                                     xmv(half, 0, i - 1), start=False, stop=False)
                    nc.tensor.matmul(ps[:], W(("f2", half, 0, 1, "h")),
                                     xmv(half, 1, i - 1), start=False, stop=True)
                if r == 4 and i < 4:
                    nc.tensor.matmul(ps[:], W(("f2", half, 4, 0, "h")),
                                     xmv(half, 0, i + 1), start=False, stop=False)
                    nc.tensor.matmul(ps[:], W(("f2", half, 4, 1, "h")),
                                     xmv(half, 1, i + 1), start=False, stop=True)
                lrelu_evac(x4[half][:, r * BCORE:(r + 1) * BCORE], ps[:], 128, BCORE)
        # build a3 window slices for H rows 5i..5i+4 (cols i*2560..)
        csl = slice(i * 5 * BCORE, (i + 1) * 5 * BCORE)
        nc.gpsimd.dma_start(a3t[0][0:120, csl], x4[0][0:120, :])
        nc.gpsimd.dma_start(a3t[1][0:32, csl], x4[0][96:128, :])
        nc.gpsimd.dma_start(a3t[1][32:120, csl], x4[1][0:88, :])
        nc.gpsimd.dma_start(a3t[2][0:96, csl], x4[1][32:128, :])

    ps_ctx.close()
    pA = ctx.enter_context(tc.tile_pool(name="pA", bufs=4, space="PSUM"))
    pB = ctx.enter_context(tc.tile_pool(name="pB", bufs=4, space="PSUM"))

    # ---- fused3 ----
    NT = [2 * 6 * (A3_JC[t][1] - A3_JC[t][0]) * 2 for t in range(3)]  # 312,312,144

    def st_ap(t, i, c):
        return a3t[t][0:a3K[t] + 1, i * BCORE + c * CH:i * BCORE + (c + 1) * CH]

    for c in range(4):
        slot = {}

        def slot_tiles(k):
            if k not in slot:
                ta = pA.tile([128, NT[0]], F32, tag="pA", name=f"sA_{c}_{k}")
                tb = pB.tile([128, NT[1] + NT[2]], F32, tag="pB", name=f"sB_{c}_{k}")
                slot[k] = (ta, tb)
            return slot[k]

        def regions(k):
            ta, tb = slot_tiles(k)
            return [ta[:, 0:NT[0]], tb[:, 0:NT[1]], tb[:, NT[1]:NT[1] + NT[2]]]

        stg = None

        def evac_slot(k):
            rlo = max(2 * k + 1, 0)
            rhi = min(2 * k + 2, 49)
            half = 0 if rhi < 25 else 1
            hb = 25 * half
            stgw = stg[:].rearrange("p (h c4 jc s) -> p h c4 jc s",
                                    h=25, c4=6, jc=32, s=2)
            for t in range(3):
                lo, hi = A3_JC[t]
                reg = regions(k)[t]
                ht = NT[t] // 2
                if 2 * k + 1 < 0:       # slot -1: row 0 only (q=1 half)
                    src = reg[:, ht:NT[t]]
                    dst = stgw[:, 0:1, :, lo:hi, :]
                elif 2 * k + 2 > 49:    # slot 24: row 49 only (q=0 half)
                    src = reg[:, 0:ht]
                    dst = stgw[:, 49 - hb:50 - hb, :, lo:hi, :]
                else:
                    src = reg[:]
                    dst = stgw[:, rlo - hb:rhi + 1 - hb, :, lo:hi, :]
                lrelu_evac(dst, src, 128, src.shape[1])

        for i in range(25):
            if i == 0:
                stg = stgp.tile([128, 9600], BF16, tag="stg", name=f"stg_{c}_0")
            for t in range(3):
                st = st_ap(t, i, c)
                # PSUM start=True marks the whole 2KB bank pending-zero, so
                # only the first write into each bank per slot-generation may
                # carry it; t=2 shares the pB bank with t=1.
                first_in_bank = t != 2
                if i == 0:
                    reg = regions(-1)[t]
                    ht = NT[t] // 2
                    nc.tensor.matmul(reg[:, ht:NT[t]], st, W(("U0", t)),
                                     start=first_in_bank, stop=True,
                                     skip_group_check=True)
                else:
                    reg = regions(i - 1)[t]
                    nc.tensor.matmul(reg[:], st, W(("U", t)),
                                     start=False, stop=True, skip_group_check=True)
                if i == 24:
                    reg = regions(24)[t]
                    ht = NT[t] // 2
                    nc.tensor.matmul(reg[:, 0:ht], st, W(("L24", t)),
                                     start=first_in_bank, stop=True,
                                     skip_group_check=True)
                else:
                    reg = regions(i)[t]
                    nc.tensor.matmul(reg[:], st, W(("L", t)),
                                     start=first_in_bank, stop=False,
                                     skip_group_check=True)
            evac_slot(i - 1)
            del slot[i - 1]
            if i - 1 == 11:  # rows 0..24 complete -> flush half 0
                sv = stg[:].rearrange("p (h c4 w) -> p h c4 w", h=25, c4=6, w=64)
                nc.sync.dma_start(out_ap[c * CH:(c + 1) * CH, 0:25, :, :], sv)
                stg = stgp.tile([128, 9600], BF16, tag="stg", name=f"stg_{c}_1")
        evac_slot(24)
        sv = stg[:].rearrange("p (h c4 w) -> p h c4 w", h=25, c4=6, w=64)
        nc.sync.dma_start(out_ap[c * CH:(c + 1) * CH, 25:50, :, :], sv)


def build_inmaps(inputs):
    import ml_dtypes
    wp, nearly = _make_packs(inputs)
    wpack = wp.build().astype(ml_dtypes.bfloat16)
    _build_program.wreg = wp.reg
    lat = np.asarray(inputs["latent"], np.float32)
    ones = np.ones((1, 12800), ml_dtypes.bfloat16)
    in_maps = []
    for i in range(NCORES):
        latT = np.ones((8, BCORE), np.float32)
        latT[:7] = lat[i * BCORE:(i + 1) * BCORE].T
        in_maps.append({"latT": latT.astype(ml_dtypes.bfloat16), "wpack": wpack,
                        "ones": ones})
    return in_maps, wpack.shape[1], nearly


def kernel(**inputs):
    inputs = {k: np.asarray(v) for k, v in inputs.items()}
    in_maps, wcols, nearly = build_inmaps(inputs)
    nc = _build_program(wcols, nearly)
    res = run_bass_kernel_spmd(nc, in_maps, core_ids=list(range(NCORES)))
    out = np.concatenate([np.asarray(res.results[i]["out"]) for i in range(NCORES)],
                         axis=0)
    return out.transpose(0, 2, 1, 3).astype(np.float32)
